# revision 1
# baseline (speedup 1.0000x reference)
"""GCN layer (PyG GCNConv semantics) on 8 Trainium2 NeuronCores via Bass.

Algorithm (per core, SPMD over 8 dst-shards of nodes):
  1. deg[n]  = 1 + sum of incoming edge weights      (vector reduce over padded slots)
  2. dinv    = rsqrt(deg)                            (DVE reciprocal + ACT sqrt)
  3. h'      = (x @ W^T) * dinv[src-shard rows]      (PE matmul + ACT scale, fp16)
  4. AllGather h' shards -> full fp16 node-feature table (256B row pitch)
  5. dma_gather (custom GPSIMD batch gather) of h'[src] for every padded
     edge slot, in 4 int16-addressable table sections
  6. msgs *= ew (fp16), segmented XY-reduce per 128-node tile,
     * dinv[dst] post-scale, + b, relu
  7. write dst-shard rows; host reassembles full [N, 64] output.

Host-side work is layout only: edge bucketing/padding by (dst tile,
table section), int conversions, node->table-row mapping, output
row de-permutation. All floating-point math runs on device.
"""

import os
import sys
import time

for _p in ("/opt/trn_rl_repo",):
    if _p not in sys.path and os.path.isdir(_p):
        sys.path.insert(0, _p)

import numpy as np

import concourse.bass as bass
import concourse.mybir as mybir
import concourse.tile as tile
from concourse import bacc
from concourse.bass_utils import run_bass_kernel_spmd

# ---------------------------------------------------------------- config

P = 128           # partitions
D = 64            # feature dim (in == out)
CORES = 8
SECS = 4          # int16-addressable table sections

MAX_PIECE_COLS = 192   # slot columns per piece (4 sections combined)


class Cfg:
    def __init__(self, n_nodes, n_cores=CORES, max_piece_cols=MAX_PIECE_COLS):
        assert n_nodes % n_cores == 0
        self.n = n_nodes
        self.cores = n_cores
        self.npc = n_nodes // n_cores                 # real nodes per core
        self.tiles = (self.npc + P - 1) // P          # 128-node tiles per core
        self.npcp = self.tiles * P                    # padded nodes per core
        self.nrows = self.npcp * n_cores              # table rows
        assert self.nrows % SECS == 0
        self.srows = self.nrows // SECS               # rows per section
        assert self.srows <= 32768, "section exceeds int16 index range"
        self.max_piece_cols = max_piece_cols

    def tau_local(self, l):
        """node local index -> table row within shard (partition-major)."""
        return (l % P) * self.tiles + (l // P)


# ---------------------------------------------------------------- host prep

def host_prep(cfg, x, edge_index, edge_weight, W, b):
    """Pure-layout preprocessing. Returns (in_maps, plan, postprocess)."""
    n, npc, npcp, T = cfg.n, cfg.npc, cfg.npcp, cfg.tiles
    C, SR = cfg.cores, cfg.srows

    src = np.asarray(edge_index[0], dtype=np.int64)
    dst = np.asarray(edge_index[1], dtype=np.int64)
    ew = np.asarray(edge_weight, dtype=np.float32)
    x = np.asarray(x, dtype=np.float32)
    W = np.asarray(W, dtype=np.float32)
    b = np.asarray(b, dtype=np.float32)

    # node -> global table row
    l_all = np.arange(npc, dtype=np.int64)
    tau_loc = (l_all % P) * T + (l_all // P)              # [npc]
    # global row for node id v: shard(v)*npcp + tau_loc[v % npc]
    def tau_global(v):
        return (v // npc) * npcp + tau_loc[v % npc]

    r_src = tau_global(src)                                # [E]
    g_src = (r_src // SR).astype(np.int64)                 # section of each edge's source row
    rloc_src = (r_src % SR).astype(np.int64)

    # self slots: every padded node (including tail pads) gets one
    # (pads have x=0 so they contribute nothing and are dropped at the end)
    selfv = np.arange(C * npcp, dtype=np.int64)            # padded node space
    core_of_pad = selfv // npcp
    l_of_pad = selfv % npcp
    tau_loc_p = (l_of_pad % P) * T + (l_of_pad // P)
    r_self = core_of_pad * npcp + tau_loc_p
    g_self = r_self // SR
    rloc_self = r_self % SR

    # counts per (padded dst node, section)
    core_e = dst // npc
    l_e = dst % npc
    dpad = core_e * npcp + l_e                             # padded dst node id
    keys_e = dpad * SECS + g_src
    cnt = np.bincount(keys_e, minlength=C * npcp * SECS).reshape(C * npcp, SECS)
    cnt[selfv, g_self] += 1                                # self slot

    # per (core, tile, section) max count over 128 nodes -> equal-K bands
    # node (c, l): tile t=l//P, partition p=l%P
    cnt4 = cnt.reshape(C, T, P, SECS)
    Ktcs = cnt4.max(axis=2)                                # [C, T, SECS]
    Kt = Ktcs.max(axis=(0, 2))                             # [T] common across cores+secs
    Kt = np.maximum(Kt, 1)

    # pieces: greedy group tiles while SECS * sum(Kt) <= max_piece_cols
    pieces = []      # list of (t0, t1, Ws)
    t0 = 0
    while t0 < T:
        t1, ws = t0, 0
        while t1 < T and SECS * (ws + Kt[t1]) <= cfg.max_piece_cols:
            ws += Kt[t1]
            t1 += 1
        assert t1 > t0, f"tile {t0} K={Kt[t0]} exceeds piece budget"
        pieces.append((t0, t1, int(ws)))
        t0 = t1
    # per-tile band offset within its piece, and piece col base
    piece_of_t = np.zeros(T, np.int64)
    base_in_piece = np.zeros(T, np.int64)
    piece_colbase = np.zeros(len(pieces), np.int64)
    colcur = 0
    for pi, (a, bnd, ws) in enumerate(pieces):
        piece_colbase[pi] = colcur
        off = 0
        for t in range(a, bnd):
            piece_of_t[t] = pi
            base_in_piece[t] = off
            off += Kt[t]
        colcur += SECS * ws
    s_cols = int(colcur)
    ws_of_t = np.array([pieces[piece_of_t[t]][2] for t in range(T)], np.int64)

    # slot column (within the ew/msgs layout) for slot (tile t, sec g, k):
    #   col = piece_colbase[piece] + g*Ws + base_in_piece[t] + k
    # index entry position within call (piece,g): i = (band_col_in_call)*128 + p
    #   band_col_in_call = base_in_piece[t] + k

    # rank of each edge within its (dpad, section) group
    order = np.lexsort((src, g_src, dpad))
    ranks = np.empty(len(src), np.int64)
    sk = (dpad * SECS + g_src)[order]
    grp_start = np.r_[0, np.nonzero(np.diff(sk))[0] + 1]
    grp_id = np.zeros(len(src), np.int64)
    grp_id[grp_start] = 1
    grp_id = np.cumsum(grp_id) - 1
    pos_in_grp = np.arange(len(src)) - grp_start[grp_id]
    ranks[order] = pos_in_grp
    # shift by one when the group also holds the self slot (self goes first)
    own = g_src == g_self[dpad]
    k_e = ranks + own.astype(np.int64)

    in_maps = []
    xt_pad = np.zeros((C, D, npcp), np.float32)
    for c in range(C):
        xs = x[c * npc:(c + 1) * npc]                       # [npc, D]
        xt_pad[c, :, :npc] = xs.T

    wt = np.ascontiguousarray(W.T)                          # [in, out]
    b128 = np.tile(b[None, :], (P, 1)).astype(np.float32)

    t_e = l_e // P
    p_e = l_e % P
    col_e = (piece_colbase[piece_of_t[t_e]] + g_src * ws_of_t[t_e]
             + base_in_piece[t_e] + k_e)
    # index entry global wrapped position:
    #   call base (piece, g) covers 128*Ws entries, stored wrapped 16-wide
    callw_e = ws_of_t[t_e]
    call_entry_base_e = piece_colbase[piece_of_t[t_e]] * P + g_src * (P * callw_e)
    i_in_call_e = (base_in_piece[t_e] + k_e) * P + p_e

    t_s = l_of_pad // P
    p_s = l_of_pad % P
    col_s = (piece_colbase[piece_of_t[t_s]] + g_self * ws_of_t[t_s]
             + base_in_piece[t_s])                           # k = 0
    callw_s = ws_of_t[t_s]
    call_entry_base_s = piece_colbase[piece_of_t[t_s]] * P + g_self * (P * callw_s)
    i_in_call_s = base_in_piece[t_s] * P + p_s

    for c in range(C):
        ew16 = np.zeros((P, s_cols), np.float16)
        idxw = np.zeros((16, s_cols * P // 16), np.int16)

        m = core_e == c
        ew16[p_e[m], col_e[m]] = ew[m].astype(np.float16)
        ie = call_entry_base_e[m] + i_in_call_e[m]
        idxw[ie % 16, ie // 16] = rloc_src[m].astype(np.int16)

        ms = core_of_pad == c
        ew16[p_s[ms], col_s[ms]] = np.float16(1.0)
        is_ = call_entry_base_s[ms] + i_in_call_s[ms]
        idxw[is_ % 16, is_ // 16] = rloc_self[ms].astype(np.int16)

        in_maps.append(dict(
            xt=xt_pad[c],
            wt=wt,
            b128=b128,
            ew=ew16,
            idxw=np.tile(idxw, (8, 1)),
        ))

    plan = dict(kt=[int(k) for k in Kt], pieces=pieces, s_cols=s_cols)

    def postprocess(results):
        full = np.empty((n, D), np.float32)
        for c in range(C):
            y = np.asarray(results[c]["y"]).reshape(npcp, D)
            full[c * npc:(c + 1) * npc] = y[tau_loc]
        return full

    return in_maps, plan, postprocess


# ---------------------------------------------------------------- device build

def _dma_gather_raw(gp, out_ap, in_ap, idxs_ap, num_idxs, elem_size, elem_step,
                    queue_num):
    """dma_gather without the 256B elem_size restriction (non-transpose HBM
    path; the ucode only requires the row STRIDE to be a 256B multiple)."""
    assert idxs_ap.dtype == mybir.dt.int16
    assert in_ap.dtype == out_ap.dtype
    stride_bytes = elem_step * mybir.dt.size(in_ap.dtype)
    assert stride_bytes % 256 == 0
    stride_256 = stride_bytes // 256
    assert 0 < stride_256 < 256
    assert num_idxs % 4 == 0 and num_idxs <= 65535
    _in_ap = gp.lower_ap_dma(in_ap, for_custom_bir_dma=True)
    _idxs_ap = gp.lower_ap(idxs_ap)
    _out_ap = gp.lower_ap(out_ap)
    return gp.add_instruction(mybir.InstDMAGatherAnt(
        name=gp.bass.get_next_instruction_name(),
        ins=[*_in_ap, _idxs_ap, gp.lower_val_access(gp.to_reg(num_idxs))],
        outs=[_out_ap],
        transpose=False,
        num_idxs=num_idxs,
        elem_size=elem_size,
        stride_bytes_256=stride_256,
        gen_mode=0,
        single_packet=False,
        queue_num=queue_num,
        sbuf_tokens_per_rank=0,
        sbuf_free_dim_per_rank=0,
        sbuf_free_dim_pad_per_rank=0,
        sbuf_byte_offset=0,
    ))


def build_program(cfg, plan, msgs_bufs=2, n_queues=1):
    T, C = cfg.tiles, cfg.cores
    npcp, nrows, SR = cfg.npcp, cfg.nrows, cfg.srows
    kt, pieces, s_cols = plan["kt"], plan["pieces"], plan["s_cols"]
    f16, f32, i16 = mybir.dt.float16, mybir.dt.float32, mybir.dt.int16
    phase = int(os.environ.get("GCN_PHASE", "9"))
    no_gather = bool(int(os.environ.get("GCN_NO_GATHER", "0")))
    full_row = bool(int(os.environ.get("GCN_FULL_ROW", "0")))
    strip = bool(int(os.environ.get("GCN_STRIP", "0")))

    nc = bacc.Bacc("TRN2", target_bir_lowering=False, debug=False,
                   enable_asserts=True, num_devices=C, num_swdge_queues=n_queues)

    xt = nc.dram_tensor("xt", [D, npcp], f32, kind="ExternalInput")
    wt = nc.dram_tensor("wt", [D, D], f32, kind="ExternalInput")
    b128 = nc.dram_tensor("b128", [P, D], f32, kind="ExternalInput")
    ewd = nc.dram_tensor("ew", [P, s_cols], f16, kind="ExternalInput")
    idxd = nc.dram_tensor("idxw", [P, s_cols * P // 16], i16, kind="ExternalInput")
    y = nc.dram_tensor("y", [npcp, D], f32, kind="ExternalOutput")

    ag_in = nc.dram_tensor("ag_in", [npcp, 2 * D], f16)
    table = nc.dram_tensor("table", [nrows, 2 * D], f16, addr_space="Shared")

    with tile.TileContext(nc) as tc:
        with (
            tc.tile_pool(name="const", bufs=1) as cp,
            tc.tile_pool(name="psum", bufs=4, space="PSUM") as pp,
            tc.tile_pool(name="mp", bufs=msgs_bufs) as mp,
            tc.tile_pool(name="ip", bufs=msgs_bufs) as ip,
        ):
            xt_sb = cp.tile([D, npcp], f32)
            wt_sb = cp.tile([D, D], f32)
            b_sb = cp.tile([P, D], f32)
            ew_sb = cp.tile([P, s_cols], f16)
            h_sb = cp.tile([P, T * 2 * D], f16)
            oacc = cp.tile([P, T * D], f32)
            deg = cp.tile([P, T], f32)
            rec = cp.tile([P, T], f32)
            dinv = cp.tile([P, T], f32)

            if phase >= 4 and not no_gather:
                from concourse import library_config
                nc.gpsimd.load_library(library_config.mlp)
            nc.vector.memset(h_sb[:], 0.0)
            if phase < 9:
                nc.vector.memset(oacc[:], 0.0)
            nc.sync.dma_start(out=xt_sb[:], in_=xt.ap())
            nc.sync.dma_start(out=wt_sb[:], in_=wt.ap())
            nc.sync.dma_start(out=b_sb[:], in_=b128.ap())
            nc.sync.dma_start(out=ew_sb[:], in_=ewd.ap())

            # ---- degree + dinv
            if phase >= 1 and not strip:
                for pi, (a, bnd, ws) in enumerate(pieces):
                    colbase = sum(SECS * pieces[q][2] for q in range(pi))
                    view = ew_sb[:, colbase:colbase + SECS * ws]
                    view = view.rearrange("p (g w) -> p g w", g=SECS)
                    off = 0
                    for t in range(a, bnd):
                        nc.vector.tensor_reduce(
                            out=deg[:, t:t + 1],
                            in_=view[:, :, off:off + kt[t]],
                            axis=mybir.AxisListType.XY,
                            op=mybir.AluOpType.add,
                        )
                        off += kt[t]
                nc.vector.reciprocal(rec[:], deg[:])
                nc.scalar.activation(dinv[:], rec[:],
                                     mybir.ActivationFunctionType.Sqrt)
                if phase < 2:
                    nc.vector.tensor_copy(out=oacc[:, 0:T], in_=dinv[:])

            # ---- h' = (x @ W^T) * dinv   (fp16 rows, 256B pitch)
            if phase >= 2 and not strip:
                for t in range(T):
                    ps = pp.tile([P, D], f32, space="PSUM")
                    nc.tensor.matmul(ps[:], lhsT=xt_sb[:, t * P:(t + 1) * P],
                                     rhs=wt_sb[:], start=True, stop=True)
                    nc.scalar.activation(
                        out=h_sb[:, t * 2 * D:t * 2 * D + D], in_=ps[:],
                        func=mybir.ActivationFunctionType.Copy,
                        scale=dinv[:, t:t + 1])
                if phase < 3:
                    nc.vector.tensor_copy(out=oacc[:, 0:T * D],
                                          in_=h_sb[:, 0:T * D])

            if phase >= 3:
                nc.sync.dma_start(
                    out=ag_in.ap().rearrange("(p t) f -> p (t f)", p=P),
                    in_=h_sb[:])
                if bool(int(os.environ.get("GCN_NO_AG", "0"))):
                    nc.gpsimd.dma_start(out=table.ap()[0:npcp, :], in_=ag_in.ap())
                else:
                    nc.gpsimd.collective_compute(
                        "AllGather", mybir.AluOpType.bypass,
                        replica_groups=[list(range(C))],
                        ins=[ag_in.ap().opt()], outs=[table.ap().opt()],
                    )

            # ---- gather + aggregate per piece
            if phase >= 4:
                for pi, (a, bnd, ws) in enumerate(pieces):
                    colbase = sum(SECS * pieces[q][2] for q in range(pi))
                    msgs = mp.tile([P, SECS * ws, D], f16, tag="msgs")
                    if full_row:
                        msgs_f = mp.tile([P, SECS * ws, 2 * D], f16, tag="msgsf")
                    idxt = ip.tile([P, SECS * ws * P // 16], i16, tag="idx")
                    nc.sync.dma_start(
                        out=idxt[:],
                        in_=idxd.ap()[:, colbase * P // 16:
                                      (colbase + SECS * ws) * P // 16])
                    max_secs = int(os.environ.get("GCN_MAX_SECS", "4"))
                    for g in range(SECS):
                        if g >= max_secs:
                            continue
                        if no_gather:
                            if g == 0:
                                nc.vector.memset(msgs[:, :, :], 0.5)
                            continue
                        if full_row:
                            sec = table.ap()[g * SR:(g + 1) * SR, :]
                            nc.gpsimd.dma_gather(
                                out_ap=msgs_f[:, g * ws:(g + 1) * ws, :],
                                in_ap=sec,
                                idxs_ap=idxt[:, g * ws * 8:(g + 1) * ws * 8],
                                num_idxs=P * ws,
                                num_idxs_reg=P * ws,
                                elem_size=2 * D,
                                single_packet=False,
                                queue_num=g % n_queues,
                            )
                        else:
                            sec = table.ap()[g * SR:(g + 1) * SR, 0:D]
                            _dma_gather_raw(
                                nc.gpsimd,
                                out_ap=msgs[:, g * ws:(g + 1) * ws, :],
                                in_ap=sec,
                                idxs_ap=idxt[:, g * ws * 8:(g + 1) * ws * 8],
                                num_idxs=P * ws,
                                elem_size=D,
                                elem_step=2 * D,
                                queue_num=g % n_queues,
                            )
                    if full_row and not no_gather:
                        nc.vector.tensor_copy(out=msgs[:, :, :],
                                              in_=msgs_f[:, :, 0:D])
                    if strip:
                        nc.vector.tensor_copy(out=oacc[:, 0:D],
                                              in_=msgs[:, 0, :])
                        continue
                    # scale by edge weights (slot scalar broadcast over feats)
                    ewp = ew_sb[:, colbase:colbase + SECS * ws]
                    nc.vector.tensor_tensor(
                        out=msgs[:, :, :], in0=msgs[:, :, :],
                        in1=ewp[:, :, None].to_broadcast([P, SECS * ws, D]),
                        op=mybir.AluOpType.mult)
                    # segmented reduce per tile, then *dinv[dst]
                    mview = msgs[:].rearrange("p (g w) f -> p f g w", g=SECS)
                    off = 0
                    for t in range(a, bnd):
                        nc.vector.tensor_reduce(
                            out=oacc[:, t * D:(t + 1) * D],
                            in_=mview[:, :, :, off:off + kt[t]],
                            axis=mybir.AxisListType.XY,
                            op=mybir.AluOpType.add,
                        )
                        nc.scalar.activation(
                            out=oacc[:, t * D:(t + 1) * D],
                            in_=oacc[:, t * D:(t + 1) * D],
                            func=mybir.ActivationFunctionType.Copy,
                            scale=dinv[:, t:t + 1])
                        off += kt[t]

            # ---- + b, relu, store
            if phase >= 5:
                ov = oacc[:].rearrange("p (t f) -> p t f", f=D)
                nc.vector.tensor_tensor(
                    out=ov, in0=ov,
                    in1=b_sb[:, None, :].to_broadcast([P, T, D]),
                    op=mybir.AluOpType.add)
                nc.scalar.activation(oacc[:], oacc[:],
                                     mybir.ActivationFunctionType.Relu)
            nc.sync.dma_start(
                out=y.ap().rearrange("(p t) f -> p (t f)", p=P),
                in_=oacc[:])

    nc.compile()
    return nc


# ---------------------------------------------------------------- runner


class _Runner:
    """Persistent PJRT executor for one compiled program: keeps the jitted
    shard_map callable so repeat calls skip retracing (mirrors
    bass2jax.run_bass_via_pjrt's multi-core path)."""

    def __init__(self, nc, n_cores):
        import jax
        from jax.experimental.shard_map import shard_map
        from jax.sharding import Mesh, PartitionSpec
        from concourse import bass2jax as B
        import concourse.mybir as mb

        B.install_neuronx_cc_hook()
        self.n_cores = n_cores
        partition_name = (nc.partition_id_tensor.name
                          if nc.partition_id_tensor else None)
        in_names, out_names, out_avals, zero_outs = [], [], [], []
        for alloc in nc.m.functions[0].allocations:
            if not isinstance(alloc, mb.MemoryLocationSet):
                continue
            name = alloc.memorylocations[0].name
            if alloc.kind == "ExternalInput":
                if name != partition_name:
                    in_names.append(name)
            elif alloc.kind == "ExternalOutput":
                shape = tuple(alloc.tensor_shape)
                dtype = mb.dt.np(alloc.dtype)
                out_names.append(name)
                out_avals.append(jax.core.ShapedArray(shape, dtype))
                zero_outs.append(np.zeros(shape, dtype))
        self.in_names = list(in_names)
        self.out_names = out_names
        self.out_avals = out_avals
        self.zero_outs = zero_outs
        n_params = len(self.in_names)
        n_outs = len(out_avals)
        all_in_names = self.in_names + out_names
        if partition_name is not None:
            all_in_names.append(partition_name)

        def _body(*args):
            operands = list(args)
            if partition_name is not None:
                operands.append(B.partition_id_tensor())
            outs = B._bass_exec_p.bind(
                *operands,
                out_avals=tuple(out_avals),
                in_names=tuple(all_in_names),
                out_names=tuple(out_names),
                lowering_input_output_aliases=(),
                sim_require_finite=True,
                sim_require_nnan=True,
                nc=nc,
            )
            return tuple(outs)

        devices = jax.devices()[:n_cores]
        mesh = Mesh(np.asarray(devices), ("core",))
        in_specs = (PartitionSpec("core"),) * (n_params + n_outs)
        out_specs = (PartitionSpec("core"),) * n_outs
        donate = tuple(range(n_params, n_params + n_outs))
        self.fn = jax.jit(
            shard_map(_body, mesh=mesh, in_specs=in_specs,
                      out_specs=out_specs, check_rep=False),
            donate_argnums=donate, keep_unused=True)

    def concat_inputs(self, in_maps):
        return [np.concatenate([np.asarray(in_maps[c][n])
                                for c in range(self.n_cores)], axis=0)
                for n in self.in_names]

    def zeros(self):
        return [np.zeros((self.n_cores * z.shape[0], *z.shape[1:]), z.dtype)
                for z in self.zero_outs]

    def call(self, concat_in):
        outs = self.fn(*concat_in, *self.zeros())
        return outs

    def run(self, in_maps):
        import jax
        outs = self.call(self.concat_inputs(in_maps))
        res = []
        for c in range(self.n_cores):
            res.append({name: np.asarray(outs[i]).reshape(
                self.n_cores, *self.out_avals[i].shape)[c]
                for i, name in enumerate(self.out_names)})
        return res


_CACHE = {}


def run(cfg, x, edge_index, edge_weight, W, b, use_sim=False):
    in_maps, plan, post = host_prep(cfg, x, edge_index, edge_weight, W, b)
    if bool(int(os.environ.get("GCN_ZERO_IDX", "0"))):
        for m in in_maps:
            m["idxw"] = np.zeros_like(m["idxw"])
    key = (cfg.n, cfg.cores, tuple(plan["kt"]),
           os.environ.get("GCN_PHASE", "9"))
    if key not in _CACHE:
        _CACHE[key] = build_program(cfg, plan)
    nc = _CACHE[key]
    if use_sim:
        from concourse import bass_interp
        sim = bass_interp.MultiCoreSim(nc, num_cores=cfg.cores)
        for c in range(cfg.cores):
            for k, v in in_maps[c].items():
                sim.cores[c].tensor(k)[:] = v
        sim.simulate(check_with_hw=False)
        results = [{"y": np.asarray(sim.cores[c].mem_tensor("y"))}
                   for c in range(cfg.cores)]
        return post(results)
    rkey = ("runner",) + key
    if rkey not in _CACHE:
        _CACHE[rkey] = _Runner(nc, cfg.cores)
    results = _CACHE[rkey].run(in_maps)
    return post(results)


def kernel(x, edge_index, edge_weight, W, b):
    cfg = Cfg(100000)
    return run(cfg, x, edge_index, edge_weight, W, b)



# revision 18
# speedup vs baseline: 3.9394x; 3.9394x over previous
"""GCN layer (PyG GCNConv semantics) on 8 Trainium2 NeuronCores via Bass.

v3 — banded gather+reduce device algorithm (proven on HW) with a
wall-clock-oriented host/transfer pipeline:

  host:   deg/dinv via one weighted bincount; edges ranked within
          (dst node, src section) groups via a single int32 radix
          argsort; slot/idx arrays built with two global scatters.
          x shipped as fp16 rows; idx shipped un-replicated ([16, L])
          and replicated to the 8 gpsimd cores on-device.
  device: h' = (x @ W^T) * dinv[src] via PE transpose + matmul (fp16),
          AllGather h' -> full node table (natural row order, 256B
          pitch), per-piece dma_gather of h'[src] into equal-K bands,
          * ew, segmented vector reduce per 128-dst tile,
          out = relu((acc + h'_own) * dinv + b)   (self loops via the
          h'_own add - no self slots), fp16 output in natural order.
"""

import os
import sys

for _p in ("/opt/trn_rl_repo",):
    if _p not in sys.path and os.path.isdir(_p):
        sys.path.insert(0, _p)

import numpy as np

import concourse.bass as bass
import concourse.mybir as mybir
import concourse.tile as tile
from concourse import bacc
from concourse import masks

# ---------------------------------------------------------------- config

P = 128           # partitions
D = 64            # feature dim (in == out)
CORES = 8
NSEC = 4          # int16-addressable table sections
MAX_PIECE_COLS = 192


class Cfg:
    def __init__(self, n_nodes, n_cores=CORES):
        assert n_nodes % n_cores == 0
        self.n = n_nodes
        self.cores = n_cores
        self.npc = n_nodes // n_cores                 # real nodes per core
        self.tiles = (self.npc + P - 1) // P          # 128-node tiles per core
        self.npcp = self.tiles * P                    # padded nodes per core
        self.nrows = self.npcp * n_cores              # table rows
        assert self.nrows % NSEC == 0
        self.srows = self.nrows // NSEC               # rows per section
        assert self.srows <= 32768, "section exceeds int16 index range"


def _pieces_from_kt(kt):
    """Greedy grouping of tiles into pieces with <= MAX_PIECE_COLS slot
    columns (NSEC * sum of widths). Deterministic; shared host/device."""
    T = len(kt)
    pieces = []
    t0 = 0
    while t0 < T:
        t1, ws = t0, 0
        while t1 < T and NSEC * (ws + kt[t1]) <= MAX_PIECE_COLS:
            ws += kt[t1]
            t1 += 1
        assert t1 > t0, f"tile {t0} K={kt[t0]} exceeds piece budget"
        pieces.append((t0, t1, int(ws)))
        t0 = t1
    return pieces


# ---------------------------------------------------------------- host prep

def host_prep(cfg, x, edge_index, edge_weight, W, b):
    """Light preprocessing. Returns (arrays dict, kt tuple). Arrays are
    concatenated across cores along axis 0 (the shard_map axis)."""
    n, npc, npcp, T = cfg.n, cfg.npc, cfg.npcp, cfg.tiles
    C, SR = cfg.cores, cfg.srows

    src = np.asarray(edge_index[0]).astype(np.int32, copy=False)
    dst = np.asarray(edge_index[1]).astype(np.int32, copy=False)
    ew = np.asarray(edge_weight, dtype=np.float32)
    x = np.asarray(x, dtype=np.float32)
    W = np.asarray(W, dtype=np.float32)
    b = np.asarray(b, dtype=np.float32)
    E = src.shape[0]

    # x rows, fp16, padded per core
    x16 = np.zeros((C, npcp, D), np.float16)
    x16[:, :npc] = x.astype(np.float16).reshape(C, npc, D)

    # deg / dinv on host
    deg = np.bincount(dst, weights=ew, minlength=n).astype(np.float32) + 1.0
    dinv = 1.0 / np.sqrt(deg)
    dv = np.zeros((C, npcp), np.float32)
    dv[:, :npc] = dinv.reshape(C, npc)
    dv = np.ascontiguousarray(dv.reshape(C, T, P).transpose(0, 2, 1))

    wt16 = np.tile(np.ascontiguousarray(W.T).astype(np.float16), (C, 1))
    b64 = np.tile(b[None, :].astype(np.float32), (C * P, 1))

    # ---- per (dst node, section) ranks
    q, l = np.divmod(src, npc)
    r_src = q * npcp + l                       # natural global table row
    g = r_src // SR
    rloc = (r_src - g * SR).astype(np.int16)
    cd, ld = np.divmod(dst, npc)
    t_e = ld >> 7
    p_e = ld & (P - 1)
    key = (cd * npcp + ld) * NSEC + g          # int32, < C*npcp*NSEC

    cnt = np.bincount(key, minlength=C * npcp * NSEC)
    kt = cnt.reshape(C, T, P, NSEC).max(axis=(0, 2, 3))
    kt = np.maximum(kt, 1)
    kt = tuple(int(v) for v in kt)

    pieces = _pieces_from_kt(kt)
    piece_of_t = np.zeros(T, np.int64)
    base_in_piece = np.zeros(T, np.int64)
    piece_colbase = np.zeros(len(pieces), np.int64)
    colcur = 0
    for pi, (a, bnd, ws) in enumerate(pieces):
        piece_colbase[pi] = colcur
        off = 0
        for t in range(a, bnd):
            piece_of_t[t] = pi
            base_in_piece[t] = off
            off += kt[t]
        colcur += NSEC * ws
    s_cols = int(colcur)
    ws_of_t = np.array([pieces[piece_of_t[t]][2] for t in range(T)], np.int64)
    colbase_t = (piece_colbase[piece_of_t] + base_in_piece).astype(np.int32)
    ws_t32 = ws_of_t.astype(np.int32)

    order = np.argsort(key, kind="stable")
    starts = np.cumsum(cnt) - cnt
    ranks = np.empty(E, np.int32)
    ranks[order] = (np.arange(E, dtype=np.int64) - starts[key[order]]).astype(
        np.int32)

    col = colbase_t[t_e] + g.astype(np.int32) * ws_t32[t_e] + ranks
    epos = col * P + p_e

    ew_slots = np.zeros((C, P, s_cols), np.float16)
    ew_slots[cd, p_e, col] = ew.astype(np.float16)
    idx_lin = np.zeros((C, s_cols * P), np.int16)
    idx_lin[cd, epos] = rloc
    idxw = np.ascontiguousarray(
        idx_lin.reshape(C, s_cols * 8, 16).transpose(0, 2, 1))

    arrays = dict(
        xr=x16.reshape(C * npcp, D),
        wt=wt16,
        b64=b64,
        dinv=dv.reshape(C * P, T),
        idxw=idxw.reshape(C * 16, s_cols * 8),
        ew=ew_slots.reshape(C * P, s_cols),
    )
    return arrays, kt


# ---------------------------------------------------------------- device build

def _dma_gather_raw(gp, out_ap, in_ap, idxs_ap, num_idxs, elem_size, elem_step,
                    queue_num):
    """dma_gather without the 256B elem_size restriction (non-transpose HBM
    path; the ucode only requires the row STRIDE to be a 256B multiple)."""
    assert idxs_ap.dtype == mybir.dt.int16
    assert in_ap.dtype == out_ap.dtype
    stride_bytes = elem_step * mybir.dt.size(in_ap.dtype)
    assert stride_bytes % 256 == 0
    stride_256 = stride_bytes // 256
    assert 0 < stride_256 < 256
    assert num_idxs % 4 == 0 and num_idxs <= 65535
    _in_ap = gp.lower_ap_dma(in_ap, for_custom_bir_dma=True)
    _idxs_ap = gp.lower_ap(idxs_ap)
    _out_ap = gp.lower_ap(out_ap)
    return gp.add_instruction(mybir.InstDMAGatherAnt(
        name=gp.bass.get_next_instruction_name(),
        ins=[*_in_ap, _idxs_ap, gp.lower_val_access(gp.to_reg(num_idxs))],
        outs=[_out_ap],
        transpose=False,
        num_idxs=num_idxs,
        elem_size=elem_size,
        stride_bytes_256=stride_256,
        gen_mode=0,
        single_packet=False,
        queue_num=queue_num,
        sbuf_tokens_per_rank=0,
        sbuf_free_dim_per_rank=0,
        sbuf_free_dim_pad_per_rank=0,
        sbuf_byte_offset=0,
    ))


def build_program(cfg, kt, n_queues=1):
    T, C = cfg.tiles, cfg.cores
    npcp, nrows, SR = cfg.npcp, cfg.nrows, cfg.srows
    pieces = _pieces_from_kt(kt)
    s_cols = NSEC * sum(ws for _, _, ws in pieces)
    f16, f32, i16 = mybir.dt.float16, mybir.dt.float32, mybir.dt.int16

    nc = bacc.Bacc("TRN2", target_bir_lowering=False, debug=False,
                   enable_asserts=True, num_devices=C, num_swdge_queues=n_queues)

    xr = nc.dram_tensor("xr", [npcp, D], f16, kind="ExternalInput")
    wt = nc.dram_tensor("wt", [D, D], f16, kind="ExternalInput")
    b64 = nc.dram_tensor("b64", [P, D], f32, kind="ExternalInput")
    dinvd = nc.dram_tensor("dinv", [P, T], f32, kind="ExternalInput")
    idxd = nc.dram_tensor("idxw", [16, s_cols * 8], i16, kind="ExternalInput")
    ewd = nc.dram_tensor("ew", [P, s_cols], f16, kind="ExternalInput")
    y = nc.dram_tensor("y", [npcp, D], f16, kind="ExternalOutput")

    ag_in = nc.dram_tensor("ag_in", [npcp, 2 * D], f16)
    table = nc.dram_tensor("table", [nrows, 2 * D], f16, addr_space="Shared")

    with tile.TileContext(nc) as tc:
        with (
            tc.tile_pool(name="const", bufs=1) as cp,
            tc.tile_pool(name="psum", bufs=4, space="PSUM") as pp,
            tc.tile_pool(name="xp", bufs=3) as xp,
            tc.tile_pool(name="mp", bufs=2) as mp,
            tc.tile_pool(name="ip", bufs=2) as ip,
        ):
            wt_sb = cp.tile([D, D], f16)
            id_sb = cp.tile([P, P], f16)
            b_sb = cp.tile([P, D], f32)
            dinv_sb = cp.tile([P, T], f32)
            ew_sb = cp.tile([P, s_cols], f16)
            h_sb = cp.tile([P, T * 2 * D], f16)
            oacc = cp.tile([P, T * D], f32)
            y_sb = cp.tile([P, T * D], f16)

            from concourse import library_config
            nc.gpsimd.load_library(library_config.mlp)

            nc.sync.dma_start(out=wt_sb[:], in_=wt.ap())
            nc.sync.dma_start(out=b_sb[:], in_=b64.ap())
            nc.sync.dma_start(out=dinv_sb[:], in_=dinvd.ap())
            nc.sync.dma_start(out=ew_sb[:], in_=ewd.ap())
            masks.make_identity(nc, id_sb[:])
            nc.vector.memset(h_sb[:], 0.0)

            # ---- h' = (x @ W^T) * dinv, fp16 rows at 256B pitch
            for t in range(T):
                xt_ld = xp.tile([P, D], f16, tag="xld")
                nc.sync.dma_start(out=xt_ld[:], in_=xr.ap()[t * P:(t + 1) * P, :])
                psT = pp.tile([D, P], f16, space="PSUM")
                nc.tensor.transpose(psT[:], xt_ld[:], id_sb[:])
                xtT = xp.tile([D, P], f16, tag="xtT")
                nc.any.tensor_copy(xtT[:], psT[:])
                psH = pp.tile([P, D], f32, space="PSUM")
                nc.tensor.matmul(psH[:], lhsT=xtT[:], rhs=wt_sb[:],
                                 start=True, stop=True)
                nc.scalar.activation(
                    out=h_sb[:, t * 2 * D:t * 2 * D + D], in_=psH[:],
                    func=mybir.ActivationFunctionType.Copy,
                    scale=dinv_sb[:, t:t + 1])

            # table rows in natural node order: row l = t*128+p
            nc.sync.dma_start(
                out=ag_in.ap().rearrange("(t p) f -> p t f", p=P),
                in_=h_sb[:].rearrange("p (t f) -> p t f", f=2 * D))
            nc.gpsimd.collective_compute(
                "AllGather", mybir.AluOpType.bypass,
                replica_groups=[list(range(C))],
                ins=[ag_in.ap().opt()], outs=[table.ap().opt()],
            )

            # ---- per piece: gather bands, * ew, segmented reduce
            for pi, (a, bnd, ws) in enumerate(pieces):
                colbase = sum(NSEC * pieces[q_][2] for q_ in range(pi))
                msgs = mp.tile([P, MAX_PIECE_COLS, D], f16, tag="msgs")
                idxt = ip.tile([P, MAX_PIECE_COLS * 8], i16, tag="idx")
                for kk in range(8):
                    nc.sync.dma_start(
                        out=idxt[16 * kk:16 * (kk + 1), 0:NSEC * ws * 8],
                        in_=idxd.ap()[:, colbase * 8:(colbase + NSEC * ws) * 8])
                for g in range(NSEC):
                    _dma_gather_raw(
                        nc.gpsimd,
                        out_ap=msgs[:, g * ws:(g + 1) * ws, :],
                        in_ap=table.ap()[g * SR:(g + 1) * SR, 0:D],
                        idxs_ap=idxt[:, g * ws * 8:(g + 1) * ws * 8],
                        num_idxs=P * ws,
                        elem_size=D,
                        elem_step=2 * D,
                        queue_num=g % n_queues,
                    )
                ewp = ew_sb[:, colbase:colbase + NSEC * ws]
                nc.vector.tensor_tensor(
                    out=msgs[:, 0:NSEC * ws, :], in0=msgs[:, 0:NSEC * ws, :],
                    in1=ewp[:, :, None].to_broadcast([P, NSEC * ws, D]),
                    op=mybir.AluOpType.mult)
                mview = msgs[:, 0:NSEC * ws, :].rearrange(
                    "p (g w) f -> p f g w", g=NSEC)
                off = 0
                for t in range(a, bnd):
                    nc.vector.tensor_reduce(
                        out=oacc[:, t * D:(t + 1) * D],
                        in_=mview[:, :, :, off:off + kt[t]],
                        axis=mybir.AxisListType.XY,
                        op=mybir.AluOpType.add,
                    )
                    off += kt[t]

            # ---- out = relu((acc + h'_own) * dinv + b)
            ov = oacc[:].rearrange("p (t f) -> p t f", f=D)
            hv = h_sb[:].rearrange("p (t f) -> p t f", f=2 * D)
            nc.vector.tensor_tensor(out=ov, in0=ov, in1=hv[:, :, 0:D],
                                    op=mybir.AluOpType.add)
            nc.vector.tensor_tensor(
                out=ov, in0=ov,
                in1=dinv_sb[:, :, None].to_broadcast([P, T, D]),
                op=mybir.AluOpType.mult)
            nc.vector.tensor_tensor(
                out=ov, in0=ov,
                in1=b_sb[:, None, :].to_broadcast([P, T, D]),
                op=mybir.AluOpType.add)
            nc.scalar.activation(y_sb[:], oacc[:],
                                 mybir.ActivationFunctionType.Relu)
            nc.sync.dma_start(
                out=y.ap().rearrange("(t p) f -> p t f", p=P),
                in_=y_sb[:].rearrange("p (t f) -> p t f", f=D))

    nc.compile()
    return nc


# ---------------------------------------------------------------- runner


class _Runner:
    """Persistent PJRT executor for one compiled program. Keeps the jitted
    shard_map callable; the donated output buffer is created on-device."""

    def __init__(self, nc, n_cores):
        import jax
        import jax.numpy as jnp
        from jax.experimental.shard_map import shard_map
        from jax.sharding import Mesh, PartitionSpec, NamedSharding
        from concourse import bass2jax as B
        import concourse.mybir as mb

        B.install_neuronx_cc_hook()
        self.n_cores = n_cores
        partition_name = (nc.partition_id_tensor.name
                          if nc.partition_id_tensor else None)
        in_names, out_names, out_avals = [], [], []
        for alloc in nc.m.functions[0].allocations:
            if not isinstance(alloc, mb.MemoryLocationSet):
                continue
            name = alloc.memorylocations[0].name
            if alloc.kind == "ExternalInput":
                if name != partition_name:
                    in_names.append(name)
            elif alloc.kind == "ExternalOutput":
                shape = tuple(alloc.tensor_shape)
                dtype = mb.dt.np(alloc.dtype)
                out_names.append(name)
                out_avals.append(jax.core.ShapedArray(shape, dtype))
        self.in_names = list(in_names)
        self.out_names = out_names
        self.out_avals = out_avals
        n_params = len(self.in_names)
        n_outs = len(out_avals)
        all_in_names = self.in_names + out_names
        if partition_name is not None:
            all_in_names.append(partition_name)

        def _body(*args):
            operands = list(args)
            if partition_name is not None:
                operands.append(B.partition_id_tensor())
            outs = B._bass_exec_p.bind(
                *operands,
                out_avals=tuple(out_avals),
                in_names=tuple(all_in_names),
                out_names=tuple(out_names),
                lowering_input_output_aliases=(),
                sim_require_finite=True,
                sim_require_nnan=True,
                nc=nc,
            )
            return tuple(outs)

        devices = jax.devices()[:n_cores]
        self.mesh = Mesh(np.asarray(devices), ("core",))
        self.sharding = NamedSharding(self.mesh, PartitionSpec("core"))
        in_specs = (PartitionSpec("core"),) * (n_params + n_outs)
        out_specs = (PartitionSpec("core"),) * n_outs
        donate = tuple(range(n_params, n_params + n_outs))
        self.fn = jax.jit(
            shard_map(_body, mesh=self.mesh, in_specs=in_specs,
                      out_specs=out_specs, check_rep=False),
            donate_argnums=donate, keep_unused=True)

        zero_shapes = tuple((n_cores * a.shape[0], *a.shape[1:])
                            for a in out_avals)
        zero_dtypes = tuple(a.dtype for a in out_avals)

        def _mk_zeros():
            return tuple(jnp.zeros(s, d)
                         for s, d in zip(zero_shapes, zero_dtypes))

        self.zeros_fn = jax.jit(_mk_zeros, out_shardings=(self.sharding,) * n_outs)

    def put(self, arr):
        import jax
        return jax.device_put(arr, self.sharding)

    def run_device(self, dev_arrays):
        outs = self.fn(*[dev_arrays[n] for n in self.in_names],
                       *self.zeros_fn())
        return {name: np.asarray(outs[i])
                for i, name in enumerate(self.out_names)}


_CACHE = {}


def _get_runner(cfg, kt):
    key = (cfg.n, cfg.cores, kt)
    rkey = ("runner",) + key
    if rkey not in _CACHE:
        if key not in _CACHE:
            _CACHE[key] = build_program(cfg, kt)
        _CACHE[rkey] = _Runner(_CACHE[key], cfg.cores)
    return _CACHE[rkey]


def run(cfg, x, edge_index, edge_weight, W, b, use_sim=False):
    C, npc, npcp = cfg.cores, cfg.npc, cfg.npcp

    arrays, kt = host_prep(cfg, x, edge_index, edge_weight, W, b)

    if use_sim:
        key = (cfg.n, cfg.cores, kt)
        if key not in _CACHE:
            _CACHE[key] = build_program(cfg, kt)
        nc = _CACHE[key]
        from concourse import bass_interp
        sim = bass_interp.MultiCoreSim(nc, num_cores=C)
        for c in range(C):
            for k, v in arrays.items():
                rows = v.shape[0] // C
                sim.cores[c].tensor(k)[:] = v[c * rows:(c + 1) * rows]
            sim.cores[c].tensor("partition_id")[:] = np.int32(c)
        sim.simulate(check_with_hw=False)
        y16 = np.stack([np.asarray(sim.cores[c].mem_tensor("y"))
                        for c in range(C)])
        full = np.empty((cfg.n, D), np.float32)
        for c in range(C):
            full[c * npc:(c + 1) * npc] = y16[c, :npc]
        return full

    runner = _get_runner(cfg, kt)
    dev = {k: runner.put(v) for k, v in arrays.items()}
    outs = runner.run_device(dev)
    y16 = outs["y"].reshape(C, npcp, D)
    full = np.empty((cfg.n, D), np.float32)
    for c in range(C):
        full[c * npc:(c + 1) * npc] = y16[c, :npc]
    return full


def kernel(x, edge_index, edge_weight, W, b):
    cfg = Cfg(100000)
    return run(cfg, x, edge_index, edge_weight, W, b)


# revision 31
# speedup vs baseline: 4.8420x; 1.2291x over previous
"""GCN layer (PyG GCNConv semantics) on 8 Trainium2 NeuronCores via Bass.

v3 — banded gather+reduce device algorithm (proven on HW) with a
wall-clock-oriented host/transfer pipeline:

  host:   deg/dinv via one weighted bincount; edges ranked within
          (dst node, src section) groups via a single int32 radix
          argsort; slot/idx arrays built with two global scatters.
          x shipped as fp16 rows; idx shipped un-replicated ([16, L])
          and replicated to the 8 gpsimd cores on-device.
  device: h' = (x @ W^T) * dinv[src] via PE transpose + matmul (fp16),
          AllGather h' -> full node table (natural row order, 256B
          pitch), per-piece dma_gather of h'[src] into equal-K bands,
          * ew, segmented vector reduce per 128-dst tile,
          out = relu((acc + h'_own) * dinv + b)   (self loops via the
          h'_own add - no self slots), fp16 output in natural order.
"""

import os
import sys

for _p in ("/opt/trn_rl_repo",):
    if _p not in sys.path and os.path.isdir(_p):
        sys.path.insert(0, _p)

import numpy as np

import concourse.bass as bass
import concourse.mybir as mybir
import concourse.tile as tile
from concourse import bacc
from concourse import masks

# ---------------------------------------------------------------- config

P = 128           # partitions
D = 64            # feature dim (in == out)
CORES = 8
NSEC = 4          # int16-addressable table sections
MAX_PIECE_COLS = 192


class Cfg:
    def __init__(self, n_nodes, n_cores=CORES):
        assert n_nodes % n_cores == 0
        self.n = n_nodes
        self.cores = n_cores
        self.npc = n_nodes // n_cores                 # real nodes per core
        self.tiles = (self.npc + P - 1) // P          # 128-node tiles per core
        self.npcp = self.tiles * P                    # padded nodes per core
        self.nrows = self.npcp * n_cores              # table rows
        assert self.nrows % NSEC == 0
        self.srows = self.nrows // NSEC               # rows per section
        assert self.srows <= 32768, "section exceeds int16 index range"


def _pieces_from_kt(kt):
    """Greedy grouping of tiles into pieces with <= MAX_PIECE_COLS slot
    columns (NSEC * sum of widths). Deterministic; shared host/device."""
    T = len(kt)
    pieces = []
    t0 = 0
    while t0 < T:
        t1, ws = t0, 0
        while t1 < T and NSEC * (ws + kt[t1]) <= MAX_PIECE_COLS:
            ws += kt[t1]
            t1 += 1
        assert t1 > t0, f"tile {t0} K={kt[t0]} exceeds piece budget"
        pieces.append((t0, t1, int(ws)))
        t0 = t1
    return pieces


# ---------------------------------------------------------------- host prep

def host_prep(cfg, x, edge_index, edge_weight, W, b, stage1_cb=None):
    """Light preprocessing. Returns (arrays dict, kt tuple). Arrays are
    concatenated across cores along axis 0 (the shard_map axis).
    stage1_cb, if given, receives the edge-independent arrays early so
    their upload can overlap the edge analysis."""
    n, npc, npcp, T = cfg.n, cfg.npc, cfg.npcp, cfg.tiles
    C, SR = cfg.cores, cfg.srows

    src = np.asarray(edge_index[0]).astype(np.int32, copy=False)
    dst = np.asarray(edge_index[1]).astype(np.int32, copy=False)
    ew = np.asarray(edge_weight, dtype=np.float32)
    x = np.asarray(x, dtype=np.float32)
    W = np.asarray(W, dtype=np.float32)
    b = np.asarray(b, dtype=np.float32)
    E = src.shape[0]

    # x rows, fp16, padded per core
    x16 = np.zeros((C, npcp, D), np.float16)
    x16[:, :npc] = x.astype(np.float16).reshape(C, npc, D)

    # deg / dinv on host
    deg = np.bincount(dst, weights=ew, minlength=n).astype(np.float32) + 1.0
    dinv = 1.0 / np.sqrt(deg)
    dv = np.zeros((C, npcp), np.float32)
    dv[:, :npc] = dinv.reshape(C, npc)
    dv = np.ascontiguousarray(dv.reshape(C, T, P).transpose(0, 2, 1))

    wt16 = np.tile(np.ascontiguousarray(W.T).astype(np.float16), (C, 1))
    b64 = np.tile(b[None, :].astype(np.float32), (C * P, 1))

    stage1 = dict(
        xr=x16.reshape(C * npcp, D),
        wt=wt16,
        b64=b64,
        dinv=dv.reshape(C * P, T),
    )
    if stage1_cb is not None:
        stage1_cb(stage1)

    # ---- per (dst node, section) ranks
    q, l = np.divmod(src, npc)
    r_src = q * npcp + l                       # natural global table row
    g = r_src // SR
    rloc = (r_src - g * SR).astype(np.int16)
    cd, ld = np.divmod(dst, npc)
    t_e = ld >> 7
    p_e = ld & (P - 1)
    key = (cd * npcp + ld) * NSEC + g          # int32, < C*npcp*NSEC

    cnt = np.bincount(key, minlength=C * npcp * NSEC)
    kt = cnt.reshape(C, T, P, NSEC).max(axis=(0, 2, 3))
    kt = np.maximum(kt, 1)
    kt = tuple(int(v) for v in kt)

    pieces = _pieces_from_kt(kt)
    piece_of_t = np.zeros(T, np.int64)
    base_in_piece = np.zeros(T, np.int64)
    piece_colbase = np.zeros(len(pieces), np.int64)
    colcur = 0
    for pi, (a, bnd, ws) in enumerate(pieces):
        piece_colbase[pi] = colcur
        off = 0
        for t in range(a, bnd):
            piece_of_t[t] = pi
            base_in_piece[t] = off
            off += kt[t]
        colcur += NSEC * ws
    s_cols = int(colcur)
    ws_of_t = np.array([pieces[piece_of_t[t]][2] for t in range(T)], np.int64)
    colbase_t = (piece_colbase[piece_of_t] + base_in_piece).astype(np.int32)
    ws_t32 = ws_of_t.astype(np.int32)

    order = np.argsort(key, kind="stable")
    starts = np.cumsum(cnt) - cnt
    ranks = np.empty(E, np.int32)
    ranks[order] = (np.arange(E, dtype=np.int64) - starts[key[order]]).astype(
        np.int32)

    col = colbase_t[t_e] + g.astype(np.int32) * ws_t32[t_e] + ranks
    epos = col * P + p_e

    ew_slots = np.zeros((C, P, s_cols), np.float16)
    ew_slots[cd, p_e, col] = ew.astype(np.float16)
    idx_lin = np.zeros((C, s_cols * P), np.int16)
    idx_lin[cd, epos] = rloc
    idxw = np.ascontiguousarray(
        idx_lin.reshape(C, s_cols * 8, 16).transpose(0, 2, 1))

    arrays = dict(
        stage1,
        idxw=idxw.reshape(C * 16, s_cols * 8),
        ew=ew_slots.reshape(C * P, s_cols),
    )
    return arrays, kt


# ---------------------------------------------------------------- device build

def _dma_gather_raw(gp, out_ap, in_ap, idxs_ap, num_idxs, elem_size, elem_step,
                    queue_num):
    """dma_gather without the 256B elem_size restriction (non-transpose HBM
    path; the ucode only requires the row STRIDE to be a 256B multiple)."""
    assert idxs_ap.dtype == mybir.dt.int16
    assert in_ap.dtype == out_ap.dtype
    stride_bytes = elem_step * mybir.dt.size(in_ap.dtype)
    assert stride_bytes % 256 == 0
    stride_256 = stride_bytes // 256
    assert 0 < stride_256 < 256
    assert num_idxs % 4 == 0 and num_idxs <= 65535
    _in_ap = gp.lower_ap_dma(in_ap, for_custom_bir_dma=True)
    _idxs_ap = gp.lower_ap(idxs_ap)
    _out_ap = gp.lower_ap(out_ap)
    return gp.add_instruction(mybir.InstDMAGatherAnt(
        name=gp.bass.get_next_instruction_name(),
        ins=[*_in_ap, _idxs_ap, gp.lower_val_access(gp.to_reg(num_idxs))],
        outs=[_out_ap],
        transpose=False,
        num_idxs=num_idxs,
        elem_size=elem_size,
        stride_bytes_256=stride_256,
        gen_mode=0,
        single_packet=False,
        queue_num=queue_num,
        sbuf_tokens_per_rank=0,
        sbuf_free_dim_per_rank=0,
        sbuf_free_dim_pad_per_rank=0,
        sbuf_byte_offset=0,
    ))


def build_program(cfg, kt, n_queues=1):
    T, C = cfg.tiles, cfg.cores
    npcp, nrows, SR = cfg.npcp, cfg.nrows, cfg.srows
    pieces = _pieces_from_kt(kt)
    s_cols = NSEC * sum(ws for _, _, ws in pieces)
    f16, f32, i16 = mybir.dt.float16, mybir.dt.float32, mybir.dt.int16

    nc = bacc.Bacc("TRN2", target_bir_lowering=False, debug=False,
                   enable_asserts=True, num_devices=C, num_swdge_queues=n_queues)

    xr = nc.dram_tensor("xr", [npcp, D], f16, kind="ExternalInput")
    wt = nc.dram_tensor("wt", [D, D], f16, kind="ExternalInput")
    b64 = nc.dram_tensor("b64", [P, D], f32, kind="ExternalInput")
    dinvd = nc.dram_tensor("dinv", [P, T], f32, kind="ExternalInput")
    idxd = nc.dram_tensor("idxw", [16, s_cols * 8], i16, kind="ExternalInput")
    ewd = nc.dram_tensor("ew", [P, s_cols], f16, kind="ExternalInput")
    y = nc.dram_tensor("y", [npcp, D], f16, kind="ExternalOutput")

    ag_in = nc.dram_tensor("ag_in", [npcp, 2 * D], f16)
    table = nc.dram_tensor("table", [nrows, 2 * D], f16, addr_space="Shared")

    with tile.TileContext(nc) as tc:
        with (
            tc.tile_pool(name="const", bufs=1) as cp,
            tc.tile_pool(name="psum", bufs=4, space="PSUM") as pp,
            tc.tile_pool(name="xp", bufs=3) as xp,
            tc.tile_pool(name="mp", bufs=2) as mp,
            tc.tile_pool(name="ip", bufs=2) as ip,
        ):
            wt_sb = cp.tile([D, D], f16)
            id_sb = cp.tile([P, P], f16)
            b_sb = cp.tile([P, D], f32)
            dinv_sb = cp.tile([P, T], f32)
            ew_sb = cp.tile([P, s_cols], f16)
            h_sb = cp.tile([P, T * 2 * D], f16)
            oacc = cp.tile([P, T * D], f32)
            y_sb = cp.tile([P, T * D], f16)

            from concourse import library_config
            nc.gpsimd.load_library(library_config.mlp)

            nc.sync.dma_start(out=wt_sb[:], in_=wt.ap())
            nc.sync.dma_start(out=b_sb[:], in_=b64.ap())
            nc.sync.dma_start(out=dinv_sb[:], in_=dinvd.ap())
            nc.sync.dma_start(out=ew_sb[:], in_=ewd.ap())
            masks.make_identity(nc, id_sb[:])
            nc.vector.memset(h_sb[:], 0.0)

            # ---- h' = (x @ W^T) * dinv, fp16 rows at 256B pitch
            for t in range(T):
                xt_ld = xp.tile([P, D], f16, tag="xld")
                nc.sync.dma_start(out=xt_ld[:], in_=xr.ap()[t * P:(t + 1) * P, :])
                psT = pp.tile([D, P], f16, space="PSUM")
                nc.tensor.transpose(psT[:], xt_ld[:], id_sb[:])
                xtT = xp.tile([D, P], f16, tag="xtT")
                nc.any.tensor_copy(xtT[:], psT[:])
                psH = pp.tile([P, D], f32, space="PSUM")
                nc.tensor.matmul(psH[:], lhsT=xtT[:], rhs=wt_sb[:],
                                 start=True, stop=True)
                nc.scalar.activation(
                    out=h_sb[:, t * 2 * D:t * 2 * D + D], in_=psH[:],
                    func=mybir.ActivationFunctionType.Copy,
                    scale=dinv_sb[:, t:t + 1])

            # table rows in natural node order: row l = t*128+p
            nc.sync.dma_start(
                out=ag_in.ap().rearrange("(t p) f -> p t f", p=P),
                in_=h_sb[:].rearrange("p (t f) -> p t f", f=2 * D))
            nc.gpsimd.collective_compute(
                "AllGather", mybir.AluOpType.bypass,
                replica_groups=[list(range(C))],
                ins=[ag_in.ap().opt()], outs=[table.ap().opt()],
            )

            # ---- per piece: gather bands, * ew, segmented reduce
            for pi, (a, bnd, ws) in enumerate(pieces):
                colbase = sum(NSEC * pieces[q_][2] for q_ in range(pi))
                msgs = mp.tile([P, MAX_PIECE_COLS, D], f16, tag="msgs")
                idxt = ip.tile([P, MAX_PIECE_COLS * 8], i16, tag="idx")
                for kk in range(8):
                    nc.sync.dma_start(
                        out=idxt[16 * kk:16 * (kk + 1), 0:NSEC * ws * 8],
                        in_=idxd.ap()[:, colbase * 8:(colbase + NSEC * ws) * 8])
                for g in range(NSEC):
                    _dma_gather_raw(
                        nc.gpsimd,
                        out_ap=msgs[:, g * ws:(g + 1) * ws, :],
                        in_ap=table.ap()[g * SR:(g + 1) * SR, 0:D],
                        idxs_ap=idxt[:, g * ws * 8:(g + 1) * ws * 8],
                        num_idxs=P * ws,
                        elem_size=D,
                        elem_step=2 * D,
                        queue_num=g % n_queues,
                    )
                ewp = ew_sb[:, colbase:colbase + NSEC * ws]
                nc.vector.tensor_tensor(
                    out=msgs[:, 0:NSEC * ws, :], in0=msgs[:, 0:NSEC * ws, :],
                    in1=ewp[:, :, None].to_broadcast([P, NSEC * ws, D]),
                    op=mybir.AluOpType.mult)
                mview = msgs[:, 0:NSEC * ws, :].rearrange(
                    "p (g w) f -> p f g w", g=NSEC)
                off = 0
                for t in range(a, bnd):
                    nc.vector.tensor_reduce(
                        out=oacc[:, t * D:(t + 1) * D],
                        in_=mview[:, :, :, off:off + kt[t]],
                        axis=mybir.AxisListType.XY,
                        op=mybir.AluOpType.add,
                    )
                    off += kt[t]

            # ---- out = relu((acc + h'_own) * dinv + b)
            ov = oacc[:].rearrange("p (t f) -> p t f", f=D)
            hv = h_sb[:].rearrange("p (t f) -> p t f", f=2 * D)
            nc.vector.tensor_tensor(out=ov, in0=ov, in1=hv[:, :, 0:D],
                                    op=mybir.AluOpType.add)
            nc.vector.tensor_tensor(
                out=ov, in0=ov,
                in1=dinv_sb[:, :, None].to_broadcast([P, T, D]),
                op=mybir.AluOpType.mult)
            nc.vector.tensor_tensor(
                out=ov, in0=ov,
                in1=b_sb[:, None, :].to_broadcast([P, T, D]),
                op=mybir.AluOpType.add)
            nc.scalar.activation(y_sb[:], oacc[:],
                                 mybir.ActivationFunctionType.Relu)
            nc.sync.dma_start(
                out=y.ap().rearrange("(t p) f -> p t f", p=P),
                in_=y_sb[:].rearrange("p (t f) -> p t f", f=D))

    nc.compile()
    return nc


# ---------------------------------------------------------------- runner


BLOB0 = ("xr", "wt", "b64", "dinv")   # edge-independent, uploaded early
BLOB1 = ("idxw", "ew")                # edge-dependent
IN_ORDER = BLOB0 + BLOB1


def blob_layout(in_names, shapes):
    """Returns [(name, blob_id, byte_off, per-shard shape, dtype)]."""
    assert tuple(in_names) == IN_ORDER, in_names
    blob_of = {n: 0 for n in BLOB0}
    blob_of.update({n: 1 for n in BLOB1})
    offs = [0, 0]
    layout = []
    for name in in_names:
        shape, dtype = shapes[name]
        bid = blob_of[name]
        nbytes = int(np.prod(shape)) * np.dtype(dtype).itemsize
        layout.append((name, bid, offs[bid], shape, dtype))
        offs[bid] += nbytes
    return layout, offs


def _pack(arrays, names, n_cores):
    """Concatenate per-core byte segments -> (n_cores, bytes) uint8."""
    segs = [np.ascontiguousarray(arrays[n]).view(np.uint8).reshape(
                n_cores, -1) for n in names]
    return np.concatenate(segs, axis=1)


_GLOBAL = {}


def _shd():
    if "shd" not in _GLOBAL:
        import jax
        from jax.sharding import Mesh, PartitionSpec, NamedSharding
        devices = jax.devices()[:CORES]
        mesh = Mesh(np.asarray(devices), ("core",))
        _GLOBAL["mesh"] = mesh
        _GLOBAL["shd"] = NamedSharding(mesh, PartitionSpec("core"))
    return _GLOBAL["shd"]


class _Runner:
    """Persistent PJRT executor for one compiled program. Inputs arrive as
    two per-core uint8 blobs (one host->device transfer each); they are
    sliced/bitcast to the kernel's tensors on-device inside shard_map.
    The donated output buffer is the previous call's output."""

    def __init__(self, nc, n_cores):
        import jax
        import jax.numpy as jnp
        from jax import lax
        from jax.experimental.shard_map import shard_map
        from jax.sharding import Mesh, PartitionSpec, NamedSharding
        from concourse import bass2jax as B
        import concourse.mybir as mb

        B.install_neuronx_cc_hook()
        self.n_cores = n_cores
        partition_name = (nc.partition_id_tensor.name
                          if nc.partition_id_tensor else None)
        in_names, out_names, out_avals = [], [], []
        shapes = {}
        for alloc in nc.m.functions[0].allocations:
            if not isinstance(alloc, mb.MemoryLocationSet):
                continue
            name = alloc.memorylocations[0].name
            if alloc.kind == "ExternalInput":
                if name != partition_name:
                    in_names.append(name)
                    shapes[name] = (tuple(alloc.tensor_shape),
                                    mb.dt.np(alloc.dtype))
            elif alloc.kind == "ExternalOutput":
                shape = tuple(alloc.tensor_shape)
                dtype = mb.dt.np(alloc.dtype)
                out_names.append(name)
                out_avals.append(jax.core.ShapedArray(shape, dtype))
        self.in_names = list(in_names)
        self.out_names = out_names
        self.out_avals = out_avals
        self.layout, self.blob_sizes = blob_layout(in_names, shapes)
        n_outs = len(out_avals)
        all_in_names = self.in_names + out_names
        if partition_name is not None:
            all_in_names.append(partition_name)

        def _body(*args):
            operands = list(args)
            if partition_name is not None:
                operands.append(B.partition_id_tensor())
            outs = B._bass_exec_p.bind(
                *operands,
                out_avals=tuple(out_avals),
                in_names=tuple(all_in_names),
                out_names=tuple(out_names),
                lowering_input_output_aliases=(),
                sim_require_finite=True,
                sim_require_nnan=True,
                nc=nc,
            )
            return tuple(outs)

        self.sharding = _shd()
        self.mesh = _GLOBAL["mesh"]
        n_params = len(self.in_names)
        in_specs = (PartitionSpec("core"),) * (n_params + n_outs)
        out_specs = (PartitionSpec("core"),) * n_outs
        self.fn = jax.jit(
            shard_map(_body, mesh=self.mesh, in_specs=in_specs,
                      out_specs=out_specs, check_rep=False),
            donate_argnums=tuple(range(n_params, n_params + n_outs)),
            keep_unused=True)

        nc_ = n_cores

        def _repack(b1, b2):
            blobs = (b1, b2)
            args = []
            for name, bid, off, shape, dtype in self.layout:
                it = np.dtype(dtype).itemsize
                nb = int(np.prod(shape)) * it
                seg = blobs[bid][:, off:off + nb]
                if it > 1:
                    seg = lax.bitcast_convert_type(
                        seg.reshape(nc_, nb // it, it), dtype)
                args.append(seg.reshape((nc_ * shape[0],) + tuple(shape[1:])))
            return tuple(args)

        self.repack_fn = jax.jit(
            _repack, out_shardings=(self.sharding,) * n_params)

        zero_shapes = tuple((n_cores * a.shape[0], *a.shape[1:])
                            for a in out_avals)
        zero_dtypes = tuple(a.dtype for a in out_avals)

        def _mk_zeros():
            return tuple(jnp.zeros(s, d)
                         for s, d in zip(zero_shapes, zero_dtypes))

        self.zeros_fn = jax.jit(_mk_zeros, out_shardings=(self.sharding,) * n_outs)
        self._ybuf = None

    def execute(self, d1, d2):
        if self._ybuf is None:
            self._ybuf = self.zeros_fn()[0]
        ins = self.repack_fn(d1, d2)
        ybuf, self._ybuf = self._ybuf, None
        outs = self.fn(*ins, ybuf)
        y_host = np.asarray(outs[0])
        self._ybuf = outs[0]
        return y_host


_CACHE = {}


def _get_runner(cfg, kt):
    key = (cfg.n, cfg.cores, kt)
    rkey = ("runner",) + key
    if rkey not in _CACHE:
        if key not in _CACHE:
            _CACHE[key] = build_program(cfg, kt)
        _CACHE[rkey] = _Runner(_CACHE[key], cfg.cores)
    return _CACHE[rkey]


def run(cfg, x, edge_index, edge_weight, W, b, use_sim=False):
    C, npc, npcp = cfg.cores, cfg.npc, cfg.npcp

    if use_sim:
        arrays, kt = host_prep(cfg, x, edge_index, edge_weight, W, b)
        key = (cfg.n, cfg.cores, kt)
        if key not in _CACHE:
            _CACHE[key] = build_program(cfg, kt)
        nc = _CACHE[key]
        from concourse import bass_interp
        sim = bass_interp.MultiCoreSim(nc, num_cores=C)
        for c in range(C):
            for k, v in arrays.items():
                rows = v.shape[0] // C
                sim.cores[c].tensor(k)[:] = v[c * rows:(c + 1) * rows]
            sim.cores[c].tensor("partition_id")[:] = np.int32(c)
        sim.simulate(check_with_hw=False)
        y16 = np.stack([np.asarray(sim.cores[c].mem_tensor("y"))
                        for c in range(C)])
        full = np.empty((cfg.n, D), np.float32)
        for c in range(C):
            full[c * npc:(c + 1) * npc] = y16[c, :npc]
        return full

    import jax
    holder = {}

    def cb(s1):
        holder["d1"] = jax.device_put(_pack(s1, BLOB0, C), _shd())

    arrays, kt = host_prep(cfg, x, edge_index, edge_weight, W, b,
                           stage1_cb=cb)
    runner = _get_runner(cfg, kt)
    d2 = jax.device_put(_pack(arrays, BLOB1, C), _shd())
    y_host = runner.execute(holder["d1"], d2)
    y16 = y_host.reshape(C, npcp, D)
    full = np.empty((cfg.n, D), np.float32)
    for c in range(C):
        full[c * npc:(c + 1) * npc] = y16[c, :npc]
    return full


def kernel(x, edge_index, edge_weight, W, b):
    cfg = Cfg(100000)
    return run(cfg, x, edge_index, edge_weight, W, b)
